# revision 1
# baseline (speedup 1.0000x reference)
"""3-layer custom GRU (original-paper variant, reset applied before the
hidden matmul) on 8 trn2 NeuronCores.

Strategy: data-parallel over batch (16 rows/core), zero collectives (the
measured per-collective cost on this stack is ~340us, which rules out any
model-parallel scheme needing per-timestep gathers). Each core runs the
full 3-layer stack on its batch shard, layer-sequentially: first a bulk
matmul computes gi_l = X_l @ Wih_l^T for all timesteps at once (X_l is x
for layer 0, else the previous layer's hidden series, kept in SBUF), then
the sequential recurrence runs over t with Whh_l^T resident in SBUF as
bf16 (fp32 psum accumulation, fp32 hidden state).

The recurrence is weight-load bound on the PE (a 128x128 bf16 stationary
block load ~53ns dominates the 16-column moving stream), so wall time is
~(Whh elements)/(128 lanes * 2.4GHz) per step regardless of batch width —
which is why replicating weights and splitting batch 8 ways is the right
trade on this machine. Layer 2's Whh^T (25.2MB bf16) exceeds the 24MB
SBUF, so its leading columns stay resident and the tail streams from HBM
every step through a small rotating buffer.

gi is staged in DRAM as [T, 16, 3H] bf16 — written contiguously by a
[(t,b), gate]-layout bulk matmul, read contiguously per step, and
transposed into [gate, batch] on the PE (identity-matmul transpose) at a
cost of ~3H/128 extra PE ops per step. tanh(v) = 2*sigmoid(2v)-1 keeps
the ACT engine on a single function table. The masked time-sum
accumulates on-chip in fp32; the host just transposes/concats the eight
per-core [3584, 16] outputs.
"""

import sys

if "/opt/trn_rl_repo" not in sys.path:
    sys.path.insert(0, "/opt/trn_rl_repo")

import numpy as np

NCORES = 8
B = 128
BC = 16                                   # batch rows per core
HS = (512, 1024, 2048)
INS = (512, 512, 1024)
KIN = tuple(i // 128 for i in INS)        # input-dim 128-chunks: 4, 4, 8
KH = tuple(h // 128 for h in HS)          # hidden-dim 128-chunks: 4, 8, 16
RES_COLS = 3584                           # resident Whh2^T columns
STR_CH = 256                              # streamed-column chunk size


def _split_multiwaits(nc):
    """walrus in this container rejects >1 sync-wait per instruction; hoist
    extras into standalone nop-waits on the same engine (per-engine program
    order is preserved, so this is semantically identical)."""
    import concourse.mybir as mybir

    for f in nc.m.functions:
        for bb in f.blocks:
            old = list(bb.instructions)
            if not any(
                ins.sync_info is not None and len(ins.sync_info.on_wait) > 1
                for ins in old
            ):
                continue
            new = []
            for ins in old:
                si = ins.sync_info
                if si is not None and len(si.on_wait) > 1:
                    waits = list(si.on_wait)
                    for j, w in enumerate(waits[:-1]):
                        new.append(
                            mybir.InstNoOp(
                                name=f"{ins.name}-ws{j}",
                                engine=ins.engine,
                                sync_info=mybir.SyncInfo(on_wait=[w], on_update=[]),
                            )
                        )
                    ins.sync_info = mybir.SyncInfo(
                        on_wait=[waits[-1]], on_update=list(si.on_update)
                    )
                new.append(ins)
            bb.instructions = new


def _build(T, waitfix=True):
    import concourse.bass as bass
    import concourse.mybir as mybir
    import concourse.tile as tile
    from concourse.masks import make_identity

    assert T % 8 == 0
    f32 = mybir.dt.float32
    bf16 = mybir.dt.bfloat16
    Sig = mybir.ActivationFunctionType.Sigmoid
    ADD = mybir.AluOpType.add
    MUL = mybir.AluOpType.mult
    NT = T * BC

    nc = bass.Bass(num_devices=NCORES)

    xT_d = nc.dram_tensor("xT", [KIN[0], 128, NT], bf16, kind="ExternalInput")
    mrep_d = nc.dram_tensor("mrep", [T, 128, BC], f32, kind="ExternalInput")
    wih_d, whh_d, bzr_d, bn2_d = [], [], [], []
    for l in range(3):
        wih_d.append(nc.dram_tensor(f"wih{l}", [KIN[l], 128, 3 * HS[l]], bf16,
                                    kind="ExternalInput"))
        whh_d.append(nc.dram_tensor(f"whh{l}", [KH[l], 128, 3 * HS[l]], bf16,
                                    kind="ExternalInput"))
        bzr_d.append(nc.dram_tensor(f"bzr{l}", [2 * HS[l] // 128, 128], f32,
                                    kind="ExternalInput"))
        bn2_d.append(nc.dram_tensor(f"bn2{l}", [HS[l] // 128, 128], f32,
                                    kind="ExternalInput"))
    out_d = nc.dram_tensor("out", [3584, BC], f32, kind="ExternalOutput")
    gi_d = [nc.dram_tensor(f"gi{l}_sc", [T, BC, 3 * HS[l]], bf16) for l in range(3)]

    with tile.TileContext(nc) as tc:
        with (
            tc.tile_pool(name="wp", bufs=1) as wp,
            tc.tile_pool(name="pb", space="PSUM", bufs=2) as pb,
            tc.tile_pool(name="pz", space="PSUM", bufs=2) as pz,
            tc.tile_pool(name="pnp", space="PSUM", bufs=2) as pnp,
            tc.tile_pool(name="pg", space="PSUM", bufs=2) as pg,
        ):
            ident = wp.tile([BC, BC], bf16, name="ident")
            make_identity(nc, ident[:])
            bzr_s, bn2_s, accs = [], [], []
            for l in range(3):
                t_ = wp.tile([128, 2 * HS[l] // 128], f32, name=f"bzr_s{l}")
                nc.sync.dma_start(out=t_[:], in_=bzr_d[l][:].rearrange("m p -> p m"))
                bzr_s.append(t_)
                t2 = wp.tile([128, HS[l] // 128], f32, name=f"bn2_s{l}")
                nc.sync.dma_start(out=t2[:], in_=bn2_d[l][:].rearrange("m p -> p m"))
                bn2_s.append(t2)
                a_ = wp.tile([128, KH[l], BC], f32, name=f"acc{l}")
                nc.vector.memset(a_[:], 0.0)
                accs.append(a_)

            def load_w(pool, dram, kc, cols, name, col0=0):
                t_ = pool.tile([128, kc, cols], bf16, name=name)
                nc.sync.dma_start(
                    out=t_[:],
                    in_=dram[:, :, col0 : col0 + cols].rearrange("k p m -> p k m"),
                )
                return t_

            def bulk_gi(l, lhs_sb, wih_s, pool):
                # gi[(t,b), gate] blocks -> DRAM [T, BC, 3H] bf16.
                # stationary: input-series chunk [128, 128 (t,b)-cols];
                # moving: Wih^T columns.
                H3 = 3 * HS[l]
                for tb in range(NT // 128):
                    t0 = tb * 128 // BC
                    for c0 in range(0, H3, 512):
                        ps = pb.tile([128, 512], f32, tag="pblk")
                        for k in range(KIN[l]):
                            nc.tensor.matmul(
                                ps[:],
                                lhs_sb[:, k, tb * 128 : (tb + 1) * 128],
                                wih_s[:, k, c0 : c0 + 512],
                                start=(k == 0),
                                stop=(k == KIN[l] - 1),
                            )
                        stg = pool.tile([128, 512], bf16, tag="stg", bufs=3)
                        nc.vector.tensor_copy(stg[:], ps[:])
                        nc.sync.dma_start(
                            out=gi_d[l][t0 : t0 + 128 // BC, :, c0 : c0 + 512]
                            .rearrange("t b n -> (t b) n"),
                            in_=stg[:],
                        )

            def recurrence(l, whh_s, h_ser, res_cols, str_dram, pool):
                kh = KH[l]
                nzr = 2 * HS[l] // 128
                nn_ = HS[l] // 128
                h3c = 3 * HS[l] // 128
                acc = accs[l]
                h_f = None
                h_bf = None
                wstr = {}

                def w_ap(t, m, k):
                    col = m * 128
                    if col < res_cols:
                        return whh_s[:, k, col : col + 128]
                    j = (col - res_cols) // STR_CH
                    if (t, j) not in wstr:
                        st = pool.tile([128, kh, STR_CH], bf16, tag=f"wstr{l}", bufs=3)
                        nc.sync.dma_start(
                            out=st[:],
                            in_=str_dram[
                                :, :, res_cols + j * STR_CH : res_cols + (j + 1) * STR_CH
                            ].rearrange("k p m -> p k m"),
                        )
                        wstr[(t, j)] = st
                    rem = (col - res_cols) % STR_CH
                    return wstr[(t, j)][:, k, rem : rem + 128]

                for t in range(T):
                    mk = pool.tile([128, BC], f32, tag=f"mk{l}", bufs=3)
                    nc.sync.dma_start(out=mk[:], in_=mrep_d[t])
                    # gi slice -> PE transpose into [gate, b] bf16 psum
                    gis = pool.tile([BC, 3 * HS[l]], bf16, tag=f"gis{l}", bufs=2)
                    nc.sync.dma_start(out=gis[:], in_=gi_d[l][t])
                    gps = pg.tile([128, h3c, BC], bf16, tag="pgi")
                    for m in range(h3c):
                        nc.tensor.matmul(
                            gps[:, m, :],
                            gis[:, m * 128 : (m + 1) * 128],
                            ident[:],
                            is_transpose=True,
                        )
                    # only one PSUM operand allowed per DVE inst; stage in SBUF
                    gsb = pool.tile([128, h3c, BC], bf16, tag=f"gsb{l}", bufs=2)
                    nc.vector.tensor_copy(gsb[:], gps[:])
                    # ---- z, r ----
                    pre = pool.tile([128, nzr, BC], f32, tag=f"pre{l}", bufs=2)
                    if t > 0:
                        ps = pz.tile([128, nzr, BC], f32, tag="pzr")
                        for m in range(nzr):
                            for k in range(kh):
                                nc.tensor.matmul(
                                    ps[:, m, :], w_ap(t, m, k), h_bf[:, k, :],
                                    start=(k == 0), stop=(k == kh - 1),
                                )
                        for m in range(nzr):
                            nc.vector.scalar_tensor_tensor(
                                pre[:, m, :], ps[:, m, :],
                                bzr_s[l][:, m : m + 1], gsb[:, m, :], ADD, ADD,
                            )
                    else:
                        for m in range(nzr):
                            nc.vector.tensor_scalar_add(
                                pre[:, m, :], gsb[:, m, :], bzr_s[l][:, m : m + 1]
                            )
                    zr = pool.tile([128, nzr, BC], f32, tag=f"zr{l}", bufs=2)
                    nc.scalar.activation(zr[:], pre[:], Sig)
                    # ---- n ----
                    pre_n = pool.tile([128, nn_, BC], f32, tag=f"pren{l}", bufs=2)
                    if t > 0:
                        rh = pool.tile([128, kh, BC], bf16, tag=f"rh{l}", bufs=2)
                        nc.vector.tensor_mul(rh[:], zr[:, nn_ : 2 * nn_, :], h_f[:])
                        ps2 = pnp.tile([128, nn_, BC], f32, tag="pn")
                        for m in range(nn_):
                            for k in range(kh):
                                nc.tensor.matmul(
                                    ps2[:, m, :], w_ap(t, nzr + m, k), rh[:, k, :],
                                    start=(k == 0), stop=(k == kh - 1),
                                )
                        nc.vector.tensor_add(
                            pre_n[:], ps2[:], gsb[:, nzr : nzr + nn_, :]
                        )
                    else:
                        nc.vector.tensor_copy(pre_n[:], gsb[:, nzr : nzr + nn_, :])
                    s_t = pool.tile([128, nn_, BC], f32, tag=f"st{l}", bufs=2)
                    for m in range(nn_):
                        nc.scalar.activation(
                            s_t[:, m, :], pre_n[:, m, :], Sig,
                            bias=bn2_s[l][:, m : m + 1], scale=2.0,
                        )
                    n_t = pool.tile([128, nn_, BC], f32, tag=f"nt{l}", bufs=2)
                    nc.vector.tensor_scalar(n_t[:], s_t[:], 2.0, -1.0, MUL, ADD)
                    # ---- h update ----
                    d = pool.tile([128, nn_, BC], f32, tag=f"d{l}", bufs=2)
                    if t > 0:
                        nc.vector.tensor_sub(d[:], h_f[:], n_t[:])
                    else:
                        nc.vector.tensor_scalar(d[:], n_t[:], -1.0, None, MUL)
                    h_new = pool.tile([128, nn_, BC], f32, tag=f"hf{l}", bufs=2)
                    nc.vector.tensor_mul(h_new[:], zr[:, 0:nn_, :], d[:])
                    nc.vector.tensor_add(h_new[:], h_new[:], n_t[:])
                    h_f = h_new
                    if h_ser is not None:
                        nc.vector.tensor_copy(h_ser[:, :, t * BC : (t + 1) * BC], h_f[:])
                        h_bf = h_ser[:, :, t * BC : (t + 1) * BC]
                    else:
                        hb = pool.tile([128, kh, BC], bf16, tag=f"hb{l}", bufs=2)
                        nc.vector.tensor_copy(hb[:], h_f[:])
                        h_bf = hb
                    # ---- masked accumulate ----
                    am = pool.tile([128, nn_, BC], f32, tag=f"am{l}", bufs=2)
                    for k in range(nn_):
                        nc.vector.tensor_mul(am[:, k, :], h_f[:, k, :], mk[:])
                    nc.vector.tensor_add(acc[:], acc[:], am[:])

            # ---------------- phases ----------------
            with tc.tile_pool(name="p_b0", bufs=1) as p_b0:
                xT = p_b0.tile([128, KIN[0], NT], bf16, name="xT_s")
                nc.sync.dma_start(out=xT[:], in_=xT_d[:].rearrange("k p m -> p k m"))
                wih0 = load_w(p_b0, wih_d[0], KIN[0], 3 * HS[0], "wih0_s")
                bulk_gi(0, xT, wih0, p_b0)

            with tc.tile_pool(name="p_s0", bufs=1) as p_s0:
                h0_ser = p_s0.tile([128, KH[0], NT], bf16, name="h0_ser")
                with tc.tile_pool(name="p_r0", bufs=1) as p_r0:
                    whh0 = load_w(p_r0, whh_d[0], KH[0], 3 * HS[0], "whh0_s")
                    recurrence(0, whh0, h0_ser, 3 * HS[0], None, p_r0)
                with tc.tile_pool(name="p_b1", bufs=1) as p_b1:
                    wih1 = load_w(p_b1, wih_d[1], KIN[1], 3 * HS[1], "wih1_s")
                    bulk_gi(1, h0_ser, wih1, p_b1)

            with tc.tile_pool(name="p_s1", bufs=1) as p_s1:
                h1_ser = p_s1.tile([128, KH[1], NT], bf16, name="h1_ser")
                with tc.tile_pool(name="p_r1", bufs=1) as p_r1:
                    whh1 = load_w(p_r1, whh_d[1], KH[1], 3 * HS[1], "whh1_s")
                    recurrence(1, whh1, h1_ser, 3 * HS[1], None, p_r1)
                with tc.tile_pool(name="p_b2", bufs=1) as p_b2:
                    wih2 = load_w(p_b2, wih_d[2], KIN[2], 3 * HS[2], "wih2_s")
                    bulk_gi(2, h1_ser, wih2, p_b2)

            with tc.tile_pool(name="p_r2", bufs=1) as p_r2:
                whh2r = load_w(p_r2, whh_d[2], KH[2], RES_COLS, "whh2r_s")
                recurrence(2, whh2r, None, RES_COLS, whh_d[2], p_r2)

            nc.sync.dma_start(
                out=out_d[0:512, :].rearrange("(k p) b -> p k b", p=128),
                in_=accs[0][:],
            )
            nc.sync.dma_start(
                out=out_d[512:1536, :].rearrange("(k p) b -> p k b", p=128),
                in_=accs[1][:],
            )
            nc.sync.dma_start(
                out=out_d[1536:3584, :].rearrange("(k p) b -> p k b", p=128),
                in_=accs[2][:],
            )

    if waitfix:
        _split_multiwaits(nc)
    return nc


# ---------------- host side ----------------

def _prep_core_inputs(c, x, mask, params, T):
    import ml_dtypes

    bf = ml_dtypes.bfloat16
    b0 = c * BC
    m = {}
    xt = x[b0 : b0 + BC, :T, :].transpose(2, 1, 0).reshape(INS[0], T * BC)
    m["xT"] = np.ascontiguousarray(xt.reshape(KIN[0], 128, T * BC)).astype(bf)
    m["mrep"] = np.ascontiguousarray(
        np.broadcast_to(mask[b0 : b0 + BC, :T].T[:, None, :], (T, 128, BC))
    ).astype(np.float32)
    for l, (Wih, Whh, bias) in enumerate(params):
        H = HS[l]
        m[f"wih{l}"] = np.ascontiguousarray(
            Wih.T.reshape(KIN[l], 128, 3 * H)
        ).astype(bf)
        m[f"whh{l}"] = np.ascontiguousarray(
            Whh.T.reshape(KH[l], 128, 3 * H)
        ).astype(bf)
        m[f"bzr{l}"] = np.ascontiguousarray(
            bias[: 2 * H].reshape(2 * H // 128, 128)
        ).astype(np.float32)
        m[f"bn2{l}"] = np.ascontiguousarray(
            (2.0 * bias[2 * H :]).reshape(H // 128, 128)
        ).astype(np.float32)
    return m


_cache = {}


def kernel(x, mask, Wih0, Whh0, b0, Wih1, Whh1, b1, Wih2, Whh2, b2):
    from concourse.bass_utils import run_bass_kernel_spmd

    x = np.asarray(x, np.float32)
    mask = np.asarray(mask, np.float32)
    T = x.shape[1]
    if T not in _cache:
        _cache[T] = _build(T)
    nc = _cache[T]
    params = [
        (np.asarray(Wih0, np.float32), np.asarray(Whh0, np.float32),
         np.asarray(b0, np.float32)),
        (np.asarray(Wih1, np.float32), np.asarray(Whh1, np.float32),
         np.asarray(b1, np.float32)),
        (np.asarray(Wih2, np.float32), np.asarray(Whh2, np.float32),
         np.asarray(b2, np.float32)),
    ]
    in_maps = [_prep_core_inputs(c, x, mask, params, T) for c in range(NCORES)]
    res = run_bass_kernel_spmd(nc, in_maps, core_ids=list(range(NCORES)))
    out = np.zeros((B, 3584), np.float32)
    for c in range(NCORES):
        out[c * BC : (c + 1) * BC, :] = res.results[c]["out"].T
    return out



# revision 7
# speedup vs baseline: 82.9358x; 82.9358x over previous
"""3-layer custom GRU (original-paper variant, reset applied before the
hidden matmul) on 8 trn2 NeuronCores.

Strategy: data-parallel over batch (16 rows/core), zero collectives. On
this axon stack the dominant per-call cost is ~40-70us per STATIC
instruction (program load), while For_i hardware-loop iterations execute
at native engine speed. So the kernel keeps the baseline dataflow but
wraps every per-timestep body in tc.For_i loops: static instruction
count drops from ~90K (fully unrolled) to ~1.5K.

Per core, layer-sequential: bulk matmul computes gi_l = X_l @ Wih_l^T
(+bias folded in) for all timesteps, then a For_i(T) recurrence with
Whh_l^T resident in SBUF as bf16 (fp32 psum, fp32 hidden state).
walrus rejects register offsets in ldweights and the engines have a
small register budget per loop body, so every dynamically-indexed slice
is staged into a fixed tile once per iteration (1-3 dynamic APs per
body) and all matmuls/DVE ops use static APs.

Layer 0's gi stays SBUF-resident in [gate, token] layout. Layers 1/2
stage gi in DRAM as [T, 16, 3H] bf16 (contiguous per-step rows) and
PE-transpose the [16, 3H] slice into [gate, batch] each step. Layer 2's
Whh^T (25.2MB bf16) exceeds SBUF: leading 3584 columns stay resident,
the 2560-column tail streams from HBM every step through two ping-pong
buffers (29us/step DMA, hidden behind ~50us of PE work).
tanh(v) = 2*sigmoid(2v)-1 keeps ACT on one function table. The masked
time-sum accumulates on-chip in fp32.
"""

import sys

if "/opt/trn_rl_repo" not in sys.path:
    sys.path.insert(0, "/opt/trn_rl_repo")

import numpy as np

NCORES = 8
B = 128
BC = 16                                   # batch rows per core
HS = (512, 1024, 2048)
INS = (512, 512, 1024)
KIN = tuple(i // 128 for i in INS)        # input-dim 128-chunks: 4, 4, 8
KH = tuple(h // 128 for h in HS)          # hidden-dim 128-chunks: 4, 8, 16
H3C = tuple(3 * h // 128 for h in HS)     # gate-dim 128-chunks: 12, 24, 48
RES_COLS = 3584                           # resident Whh2^T columns
TAIL_CH = 640                             # streamed-column chunk (5 blocks)


def _split_multiwaits(nc):
    """walrus in this container rejects >1 sync-wait per instruction; hoist
    extras into standalone nop-waits on the same engine (per-engine program
    order is preserved, so this is semantically identical)."""
    import concourse.mybir as mybir

    for f in nc.m.functions:
        for bb in f.blocks:
            old = list(bb.instructions)
            if not any(
                ins.sync_info is not None and len(ins.sync_info.on_wait) > 1
                for ins in old
            ):
                continue
            new = []
            for ins in old:
                si = ins.sync_info
                if si is not None and len(si.on_wait) > 1:
                    waits = list(si.on_wait)
                    for j, w in enumerate(waits[:-1]):
                        new.append(
                            mybir.InstNoOp(
                                name=f"{ins.name}-ws{j}",
                                engine=ins.engine,
                                sync_info=mybir.SyncInfo(on_wait=[w], on_update=[]),
                            )
                        )
                    ins.sync_info = mybir.SyncInfo(
                        on_wait=[waits[-1]], on_update=list(si.on_update)
                    )
                new.append(ins)
            bb.instructions = new


def _build(T, layers=3):
    import concourse.bass as bass
    import concourse.mybir as mybir
    import concourse.tile as tile
    from concourse.bass import ds, ts
    from concourse.masks import make_identity

    f32 = mybir.dt.float32
    bf16 = mybir.dt.bfloat16
    Sig = mybir.ActivationFunctionType.Sigmoid
    ADD = mybir.AluOpType.add
    MUL = mybir.AluOpType.mult
    NT = T * BC

    nc = bass.Bass(num_devices=NCORES)

    xT_d = nc.dram_tensor("xT", [KIN[0], 128, NT], bf16, kind="ExternalInput")
    mrep_d = nc.dram_tensor("mrep", [128, NT], f32, kind="ExternalInput")
    b0col_d = nc.dram_tensor("b0col", [128, H3C[0]], f32, kind="ExternalInput")
    brep_d = [None]
    wih_d, whh_d = [], []
    for l in range(3):
        wih_d.append(nc.dram_tensor(f"wih{l}", [KIN[l], 128, 3 * HS[l]], bf16,
                                    kind="ExternalInput"))
        whh_d.append(nc.dram_tensor(f"whh{l}", [KH[l], 128, 3 * HS[l]], bf16,
                                    kind="ExternalInput"))
        if l > 0:
            brep_d.append(nc.dram_tensor(f"brep{l}", [128, 3 * HS[l]], f32,
                                         kind="ExternalInput"))
    out_d = nc.dram_tensor("out", [3584, BC], f32, kind="ExternalOutput")
    # (t,b)-major gi staging for layers 1,2; one junk slot for prefetch overrun
    gi_d = [None,
            nc.dram_tensor("gi1_sc", [T + 1, BC, 3 * HS[1]], bf16),
            nc.dram_tensor("gi2_sc", [T + 1, BC, 3 * HS[2]], bf16)]

    with tile.TileContext(nc) as tc:
        with tc.tile_pool(name="wp", bufs=1) as wp:
            ident = wp.tile([BC, BC], bf16, name="ident")
            make_identity(nc, ident[:])
            mrep_s = wp.tile([128, NT], f32, name="mrep_s")
            nc.sync.dma_start(out=mrep_s[:], in_=mrep_d[:])
            accs = []
            for l in range(3):
                a_ = wp.tile([128, KH[l], BC], f32, name=f"acc{l}")
                nc.vector.memset(a_[:], 0.0)
                accs.append(a_)

            def load_w(pool, dram, kc, cols, name, col0=0):
                t_ = pool.tile([128, kc, cols], bf16, name=name)
                nc.sync.dma_start(
                    out=t_[:],
                    in_=dram[:, :, col0 : col0 + cols].rearrange("k p m -> p k m"),
                )
                return t_

            def recurrence_tiles(pool, l):
                nzr, nn_ = 2 * KH[l], KH[l]
                tl = {}
                tl["hf"] = pool.tile([128, KH[l], BC], f32, name=f"hf{l}")
                nc.vector.memset(tl["hf"][:], 0.0)
                tl["hbf"] = pool.tile([128, KH[l], BC], bf16, name=f"hbf{l}")
                nc.vector.memset(tl["hbf"][:], 0.0)
                tl["mk"] = pool.tile([128, BC], f32, name=f"mk{l}")
                tl["am"] = pool.tile([128, KH[l], BC], f32, name=f"am{l}")
                tl["pre"] = pool.tile([128, nzr, BC], f32, name=f"pre{l}")
                tl["zr"] = pool.tile([128, nzr, BC], f32, name=f"zr{l}")
                tl["rh"] = pool.tile([128, nn_, BC], bf16, name=f"rh{l}")
                tl["pre_n"] = pool.tile([128, nn_, BC], f32, name=f"pren{l}")
                tl["s_t"] = pool.tile([128, nn_, BC], f32, name=f"st{l}")
                tl["n_t"] = pool.tile([128, nn_, BC], f32, name=f"nt{l}")
                tl["d_t"] = pool.tile([128, nn_, BC], f32, name=f"dt{l}")
                return tl

            def step_epilogue(l, tl, pn, gi_n_ap, t, hser):
                """from n-gate preact to h update, series write, masked acc.
                dynamic APs: mask stage (1) + optional hser write (1)."""
                nzr, nn_, kh = 2 * KH[l], KH[l], KH[l]
                nc.vector.tensor_add(tl["pre_n"][:], pn[:], gi_n_ap)
                nc.scalar.activation(tl["s_t"][:], tl["pre_n"][:], Sig, scale=2.0)
                nc.vector.tensor_scalar(
                    tl["n_t"][:], tl["s_t"][:], 2.0, -1.0, MUL, ADD
                )
                nc.vector.tensor_sub(tl["d_t"][:], tl["hf"][:], tl["n_t"][:])
                nc.vector.tensor_mul(tl["hf"][:], tl["zr"][:, 0:nn_, :], tl["d_t"][:])
                nc.vector.tensor_add(tl["hf"][:], tl["hf"][:], tl["n_t"][:])
                nc.vector.tensor_copy(tl["hbf"][:], tl["hf"][:])
                if hser is not None:
                    nc.vector.tensor_copy(hser[:, :, ts(t, BC)], tl["hbf"][:])
                nc.vector.tensor_copy(tl["mk"][:], mrep_s[:, ts(t, BC)])
                for k in range(kh):
                    nc.vector.tensor_mul(
                        tl["am"][:, k, :], tl["hf"][:, k, :], tl["mk"][:]
                    )
                nc.vector.tensor_add(accs[l][:], accs[l][:], tl["am"][:])

            # ---------------- phase B0: gi0^T = Wih0^T-blocks x xT ----------
            # gi0T stays SBUF-resident: [128(gate), 12, NT] bf16, bias folded.
            with tc.tile_pool(name="p_s0", bufs=1) as p_s0:
                gi0T = p_s0.tile([128, H3C[0], NT], bf16, name="gi0T")
                with (
                    tc.tile_pool(name="p_b0", bufs=1) as p_b0,
                    tc.tile_pool(name="ps_b0", space="PSUM", bufs=2) as pb,
                ):
                    xT = p_b0.tile([128, KIN[0], NT], bf16, name="xT_s")
                    nc.sync.dma_start(out=xT[:], in_=xT_d[:].rearrange("k p m -> p k m"))
                    wih0 = load_w(p_b0, wih_d[0], KIN[0], 3 * HS[0], "wih0_s")
                    b0c = p_b0.tile([128, H3C[0]], f32, name="b0c")
                    nc.sync.dma_start(out=b0c[:], in_=b0col_d[:])
                    xcur = p_b0.tile([128, KIN[0], 512], bf16, name="xcur")
                    gstage = p_b0.tile([128, H3C[0], 512], bf16, name="gstage0")
                    with tc.For_i(0, NT // 512, name="b0c") as tk:
                        nc.vector.tensor_copy(xcur[:], xT[:, :, ts(tk, 512)])
                        for m in range(H3C[0]):
                            ps = pb.tile([128, 512], f32, tag="pblk0")
                            for k in range(KIN[0]):
                                nc.tensor.matmul(
                                    ps[:],
                                    wih0[:, k, m * 128 : (m + 1) * 128],
                                    xcur[:, k, :],
                                    start=(k == 0),
                                    stop=(k == KIN[0] - 1),
                                )
                            nc.vector.tensor_scalar_add(
                                gstage[:, m, :], ps[:], b0c[:, m : m + 1]
                            )
                        nc.vector.tensor_copy(gi0T[:, :, ts(tk, 512)], gstage[:])

                # ---------------- phase R0: layer-0 recurrence --------------
                h0ser = p_s0.tile([128, KH[0], NT], bf16, name="h0ser")
                with (
                    tc.tile_pool(name="p_r0", bufs=1) as p_r0,
                    tc.tile_pool(name="psz0", space="PSUM", bufs=1) as pzp,
                    tc.tile_pool(name="psn0", space="PSUM", bufs=1) as pnp,
                ):
                    whh0 = load_w(p_r0, whh_d[0], KH[0], 3 * HS[0], "whh0_s")
                    nzr, nn_, kh = 2 * KH[0], KH[0], KH[0]
                    tl = recurrence_tiles(p_r0, 0)
                    gcur = p_r0.tile([128, H3C[0], BC], bf16, name="gcur0")
                    with tc.For_i(0, T, name="r0t") as t:
                        nc.vector.tensor_copy(gcur[:], gi0T[:, :, ts(t, BC)])
                        pz = pzp.tile([128, nzr, BC], f32, tag="pz0")
                        for m in range(nzr):
                            for k in range(kh):
                                nc.tensor.matmul(
                                    pz[:, m, :],
                                    whh0[:, k, m * 128 : (m + 1) * 128],
                                    tl["hbf"][:, k, :],
                                    start=(k == 0), stop=(k == kh - 1),
                                )
                        nc.vector.tensor_add(tl["pre"][:], pz[:], gcur[:, 0:nzr, :])
                        nc.scalar.activation(tl["zr"][:], tl["pre"][:], Sig)
                        nc.vector.tensor_mul(
                            tl["rh"][:], tl["zr"][:, nn_:nzr, :], tl["hf"][:]
                        )
                        pn = pnp.tile([128, nn_, BC], f32, tag="pn0")
                        for m in range(nn_):
                            for k in range(kh):
                                nc.tensor.matmul(
                                    pn[:, m, :],
                                    whh0[:, k, (nzr + m) * 128 : (nzr + m + 1) * 128],
                                    tl["rh"][:, k, :],
                                    start=(k == 0), stop=(k == kh - 1),
                                )
                        step_epilogue(0, tl, pn, gcur[:, nzr:, :], t, h0ser)

                # ---------------- phase B1: gi1 = h0 x Wih1^T + b1 ----------
                if layers >= 2:
                    with (
                        tc.tile_pool(name="p_b1", bufs=1) as p_b1,
                        tc.tile_pool(name="ps_b1", space="PSUM", bufs=2) as pb,
                    ):
                        wih1 = load_w(p_b1, wih_d[1], KIN[1], 3 * HS[1], "wih1_s")
                        br1 = p_b1.tile([128, 3 * HS[1]], f32, name="br1")
                        nc.sync.dma_start(out=br1[:], in_=brep_d[1][:])
                        wcur = p_b1.tile([128, KIN[1], 512], bf16, name="wcur1")
                        brc = p_b1.tile([128, 512], f32, name="brc1")
                        gout = p_b1.tile([128, NT // 128, 512], bf16, name="gout1")
                        with tc.For_i(0, 3 * HS[1] // 512, name="b1c") as c:
                            nc.vector.tensor_copy(wcur[:], wih1[:, :, ts(c, 512)])
                            nc.vector.tensor_copy(brc[:], br1[:, ts(c, 512)])
                            for tb in range(NT // 128):
                                ps = pb.tile([128, 512], f32, tag="pblk1")
                                for k in range(KIN[1]):
                                    nc.tensor.matmul(
                                        ps[:],
                                        h0ser[:, k, tb * 128 : (tb + 1) * 128],
                                        wcur[:, k, :],
                                        start=(k == 0), stop=(k == KIN[1] - 1),
                                    )
                                nc.vector.tensor_add(gout[:, tb, :], ps[:], brc[:])
                            nc.sync.dma_start(
                                out=gi_d[1][:T]
                                .rearrange("t b n -> (t b) n")[:, ts(c, 512)]
                                .rearrange("(q p) n -> p q n", p=128),
                                in_=gout[:],
                            )
            # h0ser, gi0T freed here

            # ---------------- phase R1: layer-1 recurrence ------------------
            if layers >= 2:
                with tc.tile_pool(name="p_s1", bufs=1) as p_s1:
                    h1ser = p_s1.tile([128, KH[1], NT], bf16, name="h1ser")
                    with (
                        tc.tile_pool(name="p_r1", bufs=1) as p_r1,
                        tc.tile_pool(name="psz1", space="PSUM", bufs=1) as pzp,
                        tc.tile_pool(name="psn1", space="PSUM", bufs=1) as pnp,
                        tc.tile_pool(name="psg1", space="PSUM", bufs=1) as pgp,
                    ):
                        whh1 = load_w(p_r1, whh_d[1], KH[1], 3 * HS[1], "whh1_s")
                        nzr, nn_, kh = 2 * KH[1], KH[1], KH[1]
                        h3c = H3C[1]
                        gis = p_r1.tile([BC, 3 * HS[1]], bf16, name="gis1")
                        nc.sync.dma_start(out=gis[:], in_=gi_d[1][0])
                        gsb = p_r1.tile([128, h3c, BC], bf16, name="gsb1")
                        tl = recurrence_tiles(p_r1, 1)
                        with tc.For_i(0, T, name="r1t") as t:
                            gps = pgp.tile([128, h3c, BC], bf16, tag="pgi1")
                            for m in range(h3c):
                                nc.tensor.matmul(
                                    gps[:, m, :],
                                    gis[:, m * 128 : (m + 1) * 128],
                                    ident[:],
                                    is_transpose=True,
                                )
                            nc.vector.tensor_copy(gsb[:], gps[:])
                            # prefetch next step's gi while this step computes
                            nc.sync.dma_start(
                                out=gis[:],
                                in_=gi_d[1]
                                .rearrange("t b n -> (t b) n")[ds(t * BC + BC, BC), :],
                            )
                            pz = pzp.tile([128, nzr, BC], f32, tag="pz1")
                            for m in range(nzr):
                                for k in range(kh):
                                    nc.tensor.matmul(
                                        pz[:, m, :],
                                        whh1[:, k, m * 128 : (m + 1) * 128],
                                        tl["hbf"][:, k, :],
                                        start=(k == 0), stop=(k == kh - 1),
                                    )
                            nc.vector.tensor_add(tl["pre"][:], pz[:], gsb[:, 0:nzr, :])
                            nc.scalar.activation(tl["zr"][:], tl["pre"][:], Sig)
                            nc.vector.tensor_mul(
                                tl["rh"][:], tl["zr"][:, nn_:nzr, :], tl["hf"][:]
                            )
                            pn = pnp.tile([128, nn_, BC], f32, tag="pn1")
                            for m in range(nn_):
                                for k in range(kh):
                                    nc.tensor.matmul(
                                        pn[:, m, :],
                                        whh1[:, k, (nzr + m) * 128 : (nzr + m + 1) * 128],
                                        tl["rh"][:, k, :],
                                        start=(k == 0), stop=(k == kh - 1),
                                    )
                            step_epilogue(1, tl, pn, gsb[:, nzr:, :], t, h1ser)

                    # ------------ phase B2: gi2 = h1 x Wih2^T + b2 ----------
                    if layers >= 3:
                        with (
                            tc.tile_pool(name="p_b2", bufs=1) as p_b2,
                            tc.tile_pool(name="ps_b2", space="PSUM", bufs=2) as pb,
                        ):
                            wih2 = load_w(p_b2, wih_d[2], KIN[2], 3 * HS[2], "wih2_s")
                            br2 = p_b2.tile([128, 3 * HS[2]], f32, name="br2")
                            nc.sync.dma_start(out=br2[:], in_=brep_d[2][:])
                            wcur = p_b2.tile([128, KIN[2], 512], bf16, name="wcur2")
                            brc = p_b2.tile([128, 512], f32, name="brc2")
                            gout = p_b2.tile([128, NT // 128, 512], bf16, name="gout2")
                            with tc.For_i(0, 3 * HS[2] // 512, name="b2c") as c:
                                nc.vector.tensor_copy(wcur[:], wih2[:, :, ts(c, 512)])
                                nc.vector.tensor_copy(brc[:], br2[:, ts(c, 512)])
                                for tb in range(NT // 128):
                                    ps = pb.tile([128, 512], f32, tag="pblk2")
                                    for k in range(KIN[2]):
                                        nc.tensor.matmul(
                                            ps[:],
                                            h1ser[:, k, tb * 128 : (tb + 1) * 128],
                                            wcur[:, k, :],
                                            start=(k == 0), stop=(k == KIN[2] - 1),
                                        )
                                    nc.vector.tensor_add(gout[:, tb, :], ps[:], brc[:])
                                nc.sync.dma_start(
                                    out=gi_d[2][:T]
                                    .rearrange("t b n -> (t b) n")[:, ts(c, 512)]
                                    .rearrange("(q p) n -> p q n", p=128),
                                    in_=gout[:],
                                )

            # ---------------- phase R2: layer-2 recurrence ------------------
            if layers >= 3:
                with (
                    tc.tile_pool(name="p_r2", bufs=1) as p_r2,
                    tc.tile_pool(name="psz2", space="PSUM", bufs=1) as pzp,
                    tc.tile_pool(name="psn2", space="PSUM", bufs=1) as pnp,
                    tc.tile_pool(name="psg2", space="PSUM", bufs=1) as pgp,
                ):
                    whh2r = load_w(p_r2, whh_d[2], KH[2], RES_COLS, "whh2r_s")
                    nzr, nn_, kh = 2 * KH[2], KH[2], KH[2]
                    h3c = H3C[2]
                    wtail = [
                        p_r2.tile([128, kh, TAIL_CH], bf16, name=f"wt{j}")
                        for j in range(2)
                    ]
                    gis = p_r2.tile([BC, 3 * HS[2]], bf16, name="gis2")
                    nc.sync.dma_start(out=gis[:], in_=gi_d[2][0])
                    gsb = p_r2.tile([128, h3c, BC], bf16, name="gsb2")
                    tl = recurrence_tiles(p_r2, 2)

                    def w2_ap(m, k):
                        col = m * 128
                        if col < RES_COLS:
                            return whh2r[:, k, col : col + 128]
                        j = (col - RES_COLS) // TAIL_CH
                        rem = (col - RES_COLS) % TAIL_CH
                        return wtail[j % 2][:, k, rem : rem + 128]

                    def tail_dma(j):
                        nc.sync.dma_start(
                            out=wtail[j % 2][:],
                            in_=whh_d[2][
                                :, :, RES_COLS + j * TAIL_CH : RES_COLS + (j + 1) * TAIL_CH
                            ].rearrange("k p m -> p k m"),
                        )

                    # chunk j's first consuming block column:
                    #   j=0: 3584 (zr tail + first n block), j=1: 4224,
                    #   j=2: 4864, j=3: 5504
                    with tc.For_i(0, T, name="r2t") as t:
                        tail_dma(0)
                        gps = pgp.tile([128, h3c, BC], bf16, tag="pgi2")
                        for m in range(h3c):
                            nc.tensor.matmul(
                                gps[:, m, :],
                                gis[:, m * 128 : (m + 1) * 128],
                                ident[:],
                                is_transpose=True,
                            )
                        nc.vector.tensor_copy(gsb[:], gps[:])
                        nc.sync.dma_start(
                            out=gis[:],
                            in_=gi_d[2]
                            .rearrange("t b n -> (t b) n")[ds(t * BC + BC, BC), :],
                        )
                        tail_dma(1)
                        pz = pzp.tile([128, nzr, BC], f32, tag="pz2")
                        for m in range(nzr):
                            for k in range(kh):
                                nc.tensor.matmul(
                                    pz[:, m, :],
                                    w2_ap(m, k),
                                    tl["hbf"][:, k, :],
                                    start=(k == 0), stop=(k == kh - 1),
                                )
                        nc.vector.tensor_add(tl["pre"][:], pz[:], gsb[:, 0:nzr, :])
                        nc.scalar.activation(tl["zr"][:], tl["pre"][:], Sig)
                        nc.vector.tensor_mul(
                            tl["rh"][:], tl["zr"][:, nn_:nzr, :], tl["hf"][:]
                        )
                        pn = pnp.tile([128, nn_, BC], f32, tag="pn2")
                        for m in range(nn_):
                            col = (nzr + m) * 128
                            for k in range(kh):
                                nc.tensor.matmul(
                                    pn[:, m, :],
                                    w2_ap(nzr + m, k),
                                    tl["rh"][:, k, :],
                                    start=(k == 0), stop=(k == kh - 1),
                                )
                            # refill ping-pong buffers once their last
                            # consumer has issued (chunk0 done after col
                            # 4096; chunk1 done after col 4736)
                            if col == 4096:
                                tail_dma(2)
                            elif col == 4736:
                                tail_dma(3)
                        step_epilogue(2, tl, pn, gsb[:, nzr:, :], t, None)

            nc.sync.dma_start(
                out=out_d[0:512, :].rearrange("(k p) b -> p k b", p=128),
                in_=accs[0][:],
            )
            nc.sync.dma_start(
                out=out_d[512:1536, :].rearrange("(k p) b -> p k b", p=128),
                in_=accs[1][:],
            )
            nc.sync.dma_start(
                out=out_d[1536:3584, :].rearrange("(k p) b -> p k b", p=128),
                in_=accs[2][:],
            )

    _split_multiwaits(nc)
    return nc


# ---------------- host side ----------------

def _prep_core_inputs(c, x, mask, params, T):
    import ml_dtypes

    bf = ml_dtypes.bfloat16
    b0 = c * BC
    m = {}
    xt = x[b0 : b0 + BC, :T, :].transpose(2, 1, 0).reshape(INS[0], T * BC)
    m["xT"] = np.ascontiguousarray(xt.reshape(KIN[0], 128, T * BC)).astype(bf)
    mflat = np.ascontiguousarray(mask[b0 : b0 + BC, :T].T).reshape(T * BC)
    m["mrep"] = np.ascontiguousarray(
        np.broadcast_to(mflat[None, :], (128, T * BC))
    ).astype(np.float32)
    for l, (Wih, Whh, bias) in enumerate(params):
        H = HS[l]
        m[f"wih{l}"] = np.ascontiguousarray(
            Wih.T.reshape(KIN[l], 128, 3 * H)
        ).astype(bf)
        m[f"whh{l}"] = np.ascontiguousarray(
            Whh.T.reshape(KH[l], 128, 3 * H)
        ).astype(bf)
        if l == 0:
            m["b0col"] = np.ascontiguousarray(
                bias.reshape(H3C[0], 128).T
            ).astype(np.float32)
        else:
            m[f"brep{l}"] = np.ascontiguousarray(
                np.broadcast_to(bias[None, :], (128, 3 * H))
            ).astype(np.float32)
    return m


_cache = {}


def kernel(x, mask, Wih0, Whh0, b0, Wih1, Whh1, b1, Wih2, Whh2, b2):
    from concourse.bass_utils import run_bass_kernel_spmd

    x = np.asarray(x, np.float32)
    mask = np.asarray(mask, np.float32)
    T = x.shape[1]
    if T not in _cache:
        _cache[T] = _build(T)
    nc = _cache[T]
    params = [
        (np.asarray(Wih0, np.float32), np.asarray(Whh0, np.float32),
         np.asarray(b0, np.float32)),
        (np.asarray(Wih1, np.float32), np.asarray(Whh1, np.float32),
         np.asarray(b1, np.float32)),
        (np.asarray(Wih2, np.float32), np.asarray(Whh2, np.float32),
         np.asarray(b2, np.float32)),
    ]
    in_maps = [_prep_core_inputs(c, x, mask, params, T) for c in range(NCORES)]
    res = run_bass_kernel_spmd(nc, in_maps, core_ids=list(range(NCORES)))
    out = np.zeros((B, 3584), np.float32)
    for c in range(NCORES):
        out[c * BC : (c + 1) * BC, :] = res.results[c]["out"].T
    return out


# revision 15
# speedup vs baseline: 83.6863x; 1.0090x over previous
"""3-layer custom GRU (original-paper variant, reset applied before the
hidden matmul) on 8 trn2 NeuronCores.

Strategy: data-parallel over batch (16 rows/core), zero collectives. On
this axon stack the dominant per-call cost is ~40-70us per STATIC
instruction (program load), while For_i hardware-loop iterations execute
at native engine speed. The kernel therefore minimizes static
instruction count: every per-timestep body is a tc.For_i loop, and the
large hidden-weight matmul block sweeps (layers 1/2) are themselves
inner For_i loops whose 128x128 stationary blocks are DMA-staged from
DRAM into fixed ping-pong tiles (walrus forbids register offsets in
ldweights, so the stationary AP must be a fixed tile; the DMA source
uses the loop index).

Engine NX registers are the scarce resource (~49/engine for the whole
program; loops and every dynamically-offset AP consume them), so all
dynamic slicing is done by DMA from DRAM wherever possible (weights,
gi, mask, h-series all stage through DRAM) and the few remaining
dynamic compute APs are spread across engines.

Weight blocks live in DRAM in block-major layout [m, 128, kh, 128] so a
staged chunk is contiguous 4KB rows per partition. The block stream is
chained across loops and steps (the n-gate loop's last prefetch wraps
to the next step's first z/r chunk), so HBM weight traffic is exactly
one pass over Whh per timestep, overlapped with the PE sweep.

Per core, layer-sequential: bulk matmul computes gi_l = X_l @ Wih_l^T
(+bias folded) for all timesteps, then the For_i(T) recurrence.
Layer 0 (small weights) keeps Whh resident in SBUF with the block sweep
statically unrolled, and its gi SBUF-resident in [gate, token] layout.
Layers 1/2 stage gi in DRAM as [T, 16, 3H] bf16 and PE-transpose the
[16, 3H] slice into [gate, batch] each step. tanh(v) = 2*sigmoid(2v)-1
keeps ACT on one function table. The masked time-sum accumulates
on-chip in fp32.
"""

import sys

if "/opt/trn_rl_repo" not in sys.path:
    sys.path.insert(0, "/opt/trn_rl_repo")

import numpy as np

NCORES = 8
B = 128
BC = 16                                   # batch rows per core
HS = (512, 1024, 2048)
INS = (512, 512, 1024)
KIN = tuple(i // 128 for i in INS)        # input-dim 128-chunks: 4, 4, 8
KH = tuple(h // 128 for h in HS)          # hidden-dim 128-chunks: 4, 8, 16
H3C = tuple(3 * h // 128 for h in HS)     # gate-dim 128-chunks: 12, 24, 48
CH = 2                                    # staged blocks per DMA chunk


def _split_multiwaits(nc):
    """walrus in this container rejects >1 sync-wait per instruction; hoist
    extras into standalone nop-waits on the same engine (per-engine program
    order is preserved, so this is semantically identical)."""
    import concourse.mybir as mybir

    for f in nc.m.functions:
        for bb in f.blocks:
            old = list(bb.instructions)
            if not any(
                ins.sync_info is not None and len(ins.sync_info.on_wait) > 1
                for ins in old
            ):
                continue
            new = []
            for ins in old:
                si = ins.sync_info
                if si is not None and len(si.on_wait) > 1:
                    waits = list(si.on_wait)
                    for j, w in enumerate(waits[:-1]):
                        new.append(
                            mybir.InstNoOp(
                                name=f"{ins.name}-ws{j}",
                                engine=ins.engine,
                                sync_info=mybir.SyncInfo(on_wait=[w], on_update=[]),
                            )
                        )
                    ins.sync_info = mybir.SyncInfo(
                        on_wait=[waits[-1]], on_update=list(si.on_update)
                    )
                new.append(ins)
            bb.instructions = new


def _build(T, layers=3):
    import concourse.bass as bass
    import concourse.mybir as mybir
    import concourse.tile as tile
    from concourse.bass import ds, ts
    from concourse.masks import make_identity

    f32 = mybir.dt.float32
    bf16 = mybir.dt.bfloat16
    Sig = mybir.ActivationFunctionType.Sigmoid
    ADD = mybir.AluOpType.add
    MUL = mybir.AluOpType.mult
    NT = T * BC

    nc = bass.Bass(num_devices=NCORES)

    xT_d = nc.dram_tensor("xT", [KIN[0], 128, NT], bf16, kind="ExternalInput")
    mrep_d = nc.dram_tensor("mrep", [T, 128, BC], f32, kind="ExternalInput")
    b0col_d = nc.dram_tensor("b0col", [128, H3C[0]], f32, kind="ExternalInput")
    brep_d = [None]
    wih_d = []
    for l in range(3):
        wih_d.append(nc.dram_tensor(f"wih{l}", [KIN[l], 128, 3 * HS[l]], bf16,
                                    kind="ExternalInput"))
        if l > 0:
            brep_d.append(nc.dram_tensor(f"brep{l}", [128, 3 * HS[l]], f32,
                                         kind="ExternalInput"))
    whh0_d = nc.dram_tensor("whh0", [KH[0], 128, 3 * HS[0]], bf16,
                            kind="ExternalInput")
    # block-major hidden weights for layers 1,2: [m, 128, kh, 128]
    whhblk_d = [None]
    for l in (1, 2):
        whhblk_d.append(nc.dram_tensor(
            f"whhblk{l}", [H3C[l], 128, KH[l], 128], bf16, kind="ExternalInput"))
    out_d = nc.dram_tensor("out", [3584, BC], f32, kind="ExternalOutput")
    # (t,b)-major gi staging for layers 1,2; one junk slot for prefetch overrun
    gi_d = [None,
            nc.dram_tensor("gi1_sc", [T + 1, BC, 3 * HS[1]], bf16),
            nc.dram_tensor("gi2_sc", [T + 1, BC, 3 * HS[2]], bf16)]
    # h series round-trips through DRAM: written per step, bulk-read by B1/B2
    hser_d = [nc.dram_tensor(f"h{l}ser_sc", [T, 128, KH[l] * BC], bf16)
              for l in range(2)]

    with tile.TileContext(nc) as tc:
        with tc.tile_pool(name="wp", bufs=1) as wp:
            ident = wp.tile([BC, BC], bf16, name="ident")
            make_identity(nc, ident[:])
            accs = []
            for l in range(3):
                a_ = wp.tile([128, KH[l], BC], f32, name=f"acc{l}")
                nc.vector.memset(a_[:], 0.0)
                accs.append(a_)

            def recurrence_tiles(pool, l):
                nzr, nn_ = 2 * KH[l], KH[l]
                tl = {}
                tl["hf"] = pool.tile([128, KH[l], BC], f32, name=f"hf{l}")
                nc.vector.memset(tl["hf"][:], 0.0)
                tl["hbf"] = pool.tile([128, KH[l], BC], bf16, name=f"hbf{l}")
                nc.vector.memset(tl["hbf"][:], 0.0)
                tl["mk"] = pool.tile([128, BC], f32, name=f"mk{l}")
                tl["am"] = pool.tile([128, KH[l], BC], f32, name=f"am{l}")
                tl["pre"] = pool.tile([128, nzr, BC], f32, name=f"pre{l}")
                tl["zr"] = pool.tile([128, nzr, BC], f32, name=f"zr{l}")
                tl["rh"] = pool.tile([128, nn_, BC], bf16, name=f"rh{l}")
                tl["pre_n"] = pool.tile([128, nn_, BC], f32, name=f"pren{l}")
                tl["s_t"] = pool.tile([128, nn_, BC], f32, name=f"st{l}")
                tl["n_t"] = pool.tile([128, nn_, BC], f32, name=f"nt{l}")
                tl["d_t"] = pool.tile([128, nn_, BC], f32, name=f"dt{l}")
                return tl

            def step_epilogue(l, tl, t):
                """n preact -> tanh -> h update, series write, masked acc."""
                nn_ = KH[l]
                nc.scalar.activation(tl["s_t"][:], tl["pre_n"][:], Sig, scale=2.0)
                nc.vector.tensor_scalar(
                    tl["n_t"][:], tl["s_t"][:], 2.0, -1.0, MUL, ADD
                )
                nc.vector.tensor_sub(tl["d_t"][:], tl["hf"][:], tl["n_t"][:])
                nc.vector.tensor_mul(tl["hf"][:], tl["zr"][:, 0:nn_, :], tl["d_t"][:])
                nc.vector.tensor_add(tl["hf"][:], tl["hf"][:], tl["n_t"][:])
                nc.vector.tensor_copy(tl["hbf"][:], tl["hf"][:])
                if l < 2:
                    nc.sync.dma_start(
                        out=hser_d[l][ds(t, 1)].rearrange("o p n -> p (o n)"),
                        in_=tl["hbf"][:].rearrange("p k b -> p (k b)"),
                    )
                nc.sync.dma_start(
                    out=tl["mk"][:],
                    in_=mrep_d[ds(t, 1)].rearrange("o p b -> p (o b)"),
                )
                for k in range(KH[l]):
                    nc.vector.tensor_mul(
                        tl["am"][:, k, :], tl["hf"][:, k, :], tl["mk"][:]
                    )
                nc.vector.tensor_add(accs[l][:], accs[l][:], tl["am"][:])

            def staged_sweep(l, st, psp, blk0, nblk, moving, dest, loop_name):
                """Sweep `nblk` stationary blocks starting at block `blk0` of
                whhblk_d[l], staged through ping-pong tiles st[0]/st[1] (CH
                blocks each).  Matmuls accumulate into a static psum window;
                one DVE copy per body routes the window into `dest` slots.
                Prefetch is chained: the loop assumes its first 2*CH blocks
                are staged and leaves the 2*CH blocks after its range staged
                (wrapping at the layer's total block count)."""
                kh = KH[l]
                wrap = H3C[l]
                stride = 2 * CH
                n_it = nblk // stride

                def body(j, const):
                    pzw = psp.tile([128, stride, BC], f32, tag=f"pw_{loop_name}")
                    for buf in range(2):
                        for bi in range(CH):
                            slot = buf * CH + bi
                            for k in range(kh):
                                nc.tensor.matmul(
                                    pzw[:, slot, :],
                                    st[buf][:, bi, k, :],
                                    moving[:, k, :],
                                    start=(k == 0), stop=(k == kh - 1),
                                )
                        pre_off = (blk0 + j * stride + buf * CH + stride)
                        if const:
                            pre_off = pre_off % wrap
                        else:
                            pre_off = pre_off % wrap  # ScalarValue mod
                        dma_eng = nc.scalar if not const else nc.sync
                        dma_eng.dma_start(
                            out=st[buf][:],
                            in_=whhblk_d[l]
                            .rearrange("m p k c -> p m k c")[
                                :, ds(pre_off, CH), :, :
                            ],
                        )
                    nc.vector.tensor_copy(
                        dest[:, ds(j * stride, stride), :], pzw[:]
                    )

                if n_it <= 4:
                    for j in range(n_it):
                        body(j, True)
                else:
                    with tc.For_i(0, n_it, name=loop_name) as j:
                        body(j, False)

            def stage_tiles(pool, l):
                st = [
                    pool.tile([128, CH, KH[l], 128], bf16, name=f"stg{l}_{j}")
                    for j in range(2)
                ]
                for j in range(2):
                    nc.sync.dma_start(
                        out=st[j][:],
                        in_=whhblk_d[l]
                        .rearrange("m p k c -> p m k c")[:, j * CH : (j + 1) * CH, :, :],
                    )
                return st

            def load_hser(pool, l):
                t_ = pool.tile([128, KH[l], NT], bf16, name=f"h{l}ser_s")
                nc.sync.dma_start(
                    out=t_[:].rearrange("p k (t b) -> p k t b", t=T),
                    in_=hser_d[l].rearrange("t p (k b) -> p k t b", k=KH[l]),
                )
                return t_

            def bulk_tokmajor(l, pb, pool, hsrc):
                """gi_l = hsrc-tokens x Wih_l^T + b_l  ->  gi_d[l] (t,b)-major.
                Stationary = hsrc token-blocks (static APs); moving = Wih
                column chunks DMA-staged from DRAM per For_i iteration."""
                wcur = pool.tile([128, KIN[l], 512], bf16, name=f"wcur{l}")
                brc = pool.tile([128, 512], f32, name=f"brc{l}")
                gout = pool.tile([128, NT // 128, 512], bf16, name=f"gout{l}")
                with tc.For_i(0, 3 * HS[l] // 512, name=f"b{l}c") as c:
                    nc.scalar.dma_start(
                        out=wcur[:],
                        in_=wih_d[l][:].rearrange("k p m -> p k m")[:, :, ts(c, 512)],
                    )
                    nc.scalar.dma_start(out=brc[:], in_=brep_d[l][:, ts(c, 512)])
                    for tb in range(NT // 128):
                        ps = pb.tile([128, 512], f32, tag=f"pblk{l}")
                        for k in range(KIN[l]):
                            nc.tensor.matmul(
                                ps[:],
                                hsrc[:, k, tb * 128 : (tb + 1) * 128],
                                wcur[:, k, :],
                                start=(k == 0), stop=(k == KIN[l] - 1),
                            )
                        nc.vector.tensor_add(gout[:, tb, :], ps[:], brc[:])
                    nc.sync.dma_start(
                        out=gi_d[l][:T]
                        .rearrange("t b n -> (t b) n")[:, ts(c, 512)]
                        .rearrange("(q p) n -> p q n", p=128),
                        in_=gout[:],
                    )

            # ---------------- phase B0: gi0^T = Wih0^T-blocks x xT ----------
            # gi0T stays SBUF-resident: [128(gate), 12, NT] bf16, bias folded.
            with tc.tile_pool(name="p_s0", bufs=1) as p_s0:
                gi0T = p_s0.tile([128, H3C[0], NT], bf16, name="gi0T")
                with (
                    tc.tile_pool(name="p_b0", bufs=1) as p_b0,
                    tc.tile_pool(name="ps_b0", space="PSUM", bufs=2) as pb,
                ):
                    wih0 = p_b0.tile([128, KIN[0], 3 * HS[0]], bf16, name="wih0_s")
                    nc.sync.dma_start(
                        out=wih0[:], in_=wih_d[0][:].rearrange("k p m -> p k m")
                    )
                    b0c = p_b0.tile([128, H3C[0]], f32, name="b0c")
                    nc.sync.dma_start(out=b0c[:], in_=b0col_d[:])
                    xcur = p_b0.tile([128, KIN[0], 512], bf16, name="xcur")
                    gstage = p_b0.tile([128, H3C[0], 512], bf16, name="gstage0")
                    with tc.For_i(0, NT // 512, name="b0c") as tk:
                        nc.scalar.dma_start(
                            out=xcur[:],
                            in_=xT_d[:].rearrange("k p m -> p k m")[:, :, ts(tk, 512)],
                        )
                        for m in range(H3C[0]):
                            ps = pb.tile([128, 512], f32, tag="pblk0")
                            for k in range(KIN[0]):
                                nc.tensor.matmul(
                                    ps[:],
                                    wih0[:, k, m * 128 : (m + 1) * 128],
                                    xcur[:, k, :],
                                    start=(k == 0),
                                    stop=(k == KIN[0] - 1),
                                )
                            nc.vector.tensor_scalar_add(
                                gstage[:, m, :], ps[:], b0c[:, m : m + 1]
                            )
                        nc.vector.tensor_copy(gi0T[:, :, ts(tk, 512)], gstage[:])

                # ---------------- phase R0: layer-0 recurrence --------------
                # small weights: Whh0 SBUF-resident, block sweep unrolled
                with (
                    tc.tile_pool(name="p_r0", bufs=1) as p_r0,
                    tc.tile_pool(name="psz0", space="PSUM", bufs=1) as pzp,
                    tc.tile_pool(name="psn0", space="PSUM", bufs=1) as pnp,
                ):
                    whh0 = p_r0.tile([128, KH[0], 3 * HS[0]], bf16, name="whh0_s")
                    nc.sync.dma_start(
                        out=whh0[:], in_=whh0_d[:].rearrange("k p m -> p k m")
                    )
                    nzr, nn_, kh = 2 * KH[0], KH[0], KH[0]
                    tl = recurrence_tiles(p_r0, 0)
                    gcur = p_r0.tile([128, H3C[0], BC], bf16, name="gcur0")
                    with tc.For_i(0, T, name="r0t") as t:
                        nc.vector.tensor_copy(gcur[:], gi0T[:, :, ts(t, BC)])
                        pz = pzp.tile([128, nzr, BC], f32, tag="pz0")
                        for m in range(nzr):
                            for k in range(kh):
                                nc.tensor.matmul(
                                    pz[:, m, :],
                                    whh0[:, k, m * 128 : (m + 1) * 128],
                                    tl["hbf"][:, k, :],
                                    start=(k == 0), stop=(k == kh - 1),
                                )
                        nc.vector.tensor_add(tl["pre"][:], pz[:], gcur[:, 0:nzr, :])
                        nc.scalar.activation(tl["zr"][:], tl["pre"][:], Sig)
                        nc.vector.tensor_mul(
                            tl["rh"][:], tl["zr"][:, nn_:nzr, :], tl["hf"][:]
                        )
                        pn = pnp.tile([128, nn_, BC], f32, tag="pn0")
                        for m in range(nn_):
                            for k in range(kh):
                                nc.tensor.matmul(
                                    pn[:, m, :],
                                    whh0[:, k, (nzr + m) * 128 : (nzr + m + 1) * 128],
                                    tl["rh"][:, k, :],
                                    start=(k == 0), stop=(k == kh - 1),
                                )
                        nc.vector.tensor_add(
                            tl["pre_n"][:], pn[:], gcur[:, nzr : nzr + nn_, :]
                        )
                        step_epilogue(0, tl, t)

            # ---------------- phase B1: gi1 = h0 x Wih1^T + b1 --------------
            if layers >= 2:
                with (
                    tc.tile_pool(name="p_b1", bufs=1) as p_b1,
                    tc.tile_pool(name="ps_b1", space="PSUM", bufs=2) as pb,
                ):
                    h0ser = load_hser(p_b1, 0)
                    bulk_tokmajor(1, pb, p_b1, h0ser)

                # ---------------- phase R1: layer-1 recurrence --------------
                with (
                    tc.tile_pool(name="p_r1", bufs=1) as p_r1,
                    tc.tile_pool(name="psz1", space="PSUM", bufs=1) as pzp,
                    tc.tile_pool(name="psn1", space="PSUM", bufs=1) as pnp,
                    tc.tile_pool(name="psg1", space="PSUM", bufs=1) as pgp,
                ):
                    nzr, nn_, kh = 2 * KH[1], KH[1], KH[1]
                    h3c = H3C[1]
                    st = stage_tiles(p_r1, 1)
                    gis = p_r1.tile([BC, 3 * HS[1]], bf16, name="gis1")
                    nc.sync.dma_start(out=gis[:], in_=gi_d[1][0])
                    gsb = p_r1.tile([128, h3c, BC], bf16, name="gsb1")
                    tl = recurrence_tiles(p_r1, 1)
                    with tc.For_i(0, T, name="r1t") as t:
                        gps = pgp.tile([128, h3c, BC], bf16, tag="pgi1")
                        for m in range(h3c):
                            nc.tensor.matmul(
                                gps[:, m, :],
                                gis[:, m * 128 : (m + 1) * 128],
                                ident[:],
                                is_transpose=True,
                            )
                        nc.vector.tensor_copy(gsb[:], gps[:])
                        # prefetch next step's gi while this step computes
                        nc.sync.dma_start(
                            out=gis[:],
                            in_=gi_d[1]
                            .rearrange("t b n -> (t b) n")[ds(t * BC + BC, BC), :],
                        )
                        staged_sweep(1, st, pzp, 0, nzr, tl["hbf"], tl["pre"],
                                     "r1zr")
                        nc.vector.tensor_add(tl["pre"][:], tl["pre"][:],
                                             gsb[:, 0:nzr, :])
                        nc.scalar.activation(tl["zr"][:], tl["pre"][:], Sig)
                        nc.vector.tensor_mul(
                            tl["rh"][:], tl["zr"][:, nn_:nzr, :], tl["hf"][:]
                        )
                        staged_sweep(1, st, pnp, nzr, nn_, tl["rh"], tl["pre_n"],
                                     "r1n")
                        nc.vector.tensor_add(tl["pre_n"][:], tl["pre_n"][:],
                                             gsb[:, nzr:, :])
                        step_epilogue(1, tl, t)

            # ---------------- phase B2: gi2 = h1 x Wih2^T + b2 --------------
            if layers >= 3:
                with (
                    tc.tile_pool(name="p_b2", bufs=1) as p_b2,
                    tc.tile_pool(name="ps_b2", space="PSUM", bufs=2) as pb,
                ):
                    h1ser = load_hser(p_b2, 1)
                    bulk_tokmajor(2, pb, p_b2, h1ser)

                # ---------------- phase R2: layer-2 recurrence --------------
                with (
                    tc.tile_pool(name="p_r2", bufs=1) as p_r2,
                    tc.tile_pool(name="psz2", space="PSUM", bufs=1) as pzp,
                    tc.tile_pool(name="psn2", space="PSUM", bufs=1) as pnp,
                    tc.tile_pool(name="psg2", space="PSUM", bufs=1) as pgp,
                ):
                    nzr, nn_, kh = 2 * KH[2], KH[2], KH[2]
                    h3c = H3C[2]
                    st = stage_tiles(p_r2, 2)
                    gis = p_r2.tile([BC, 3 * HS[2]], bf16, name="gis2")
                    nc.sync.dma_start(out=gis[:], in_=gi_d[2][0])
                    gsb = p_r2.tile([128, h3c, BC], bf16, name="gsb2")
                    tl = recurrence_tiles(p_r2, 2)
                    with tc.For_i(0, T, name="r2t") as t:
                        gps = pgp.tile([128, h3c, BC], bf16, tag="pgi2")
                        for m in range(h3c):
                            nc.tensor.matmul(
                                gps[:, m, :],
                                gis[:, m * 128 : (m + 1) * 128],
                                ident[:],
                                is_transpose=True,
                            )
                        nc.vector.tensor_copy(gsb[:], gps[:])
                        nc.sync.dma_start(
                            out=gis[:],
                            in_=gi_d[2]
                            .rearrange("t b n -> (t b) n")[ds(t * BC + BC, BC), :],
                        )
                        staged_sweep(2, st, pzp, 0, nzr, tl["hbf"], tl["pre"],
                                     "r2zr")
                        nc.vector.tensor_add(tl["pre"][:], tl["pre"][:],
                                             gsb[:, 0:nzr, :])
                        nc.scalar.activation(tl["zr"][:], tl["pre"][:], Sig)
                        nc.vector.tensor_mul(
                            tl["rh"][:], tl["zr"][:, nn_:nzr, :], tl["hf"][:]
                        )
                        staged_sweep(2, st, pnp, nzr, nn_, tl["rh"], tl["pre_n"],
                                     "r2n")
                        nc.vector.tensor_add(tl["pre_n"][:], tl["pre_n"][:],
                                             gsb[:, nzr:, :])
                        step_epilogue(2, tl, t)

            nc.sync.dma_start(
                out=out_d[0:512, :].rearrange("(k p) b -> p k b", p=128),
                in_=accs[0][:],
            )
            nc.sync.dma_start(
                out=out_d[512:1536, :].rearrange("(k p) b -> p k b", p=128),
                in_=accs[1][:],
            )
            nc.sync.dma_start(
                out=out_d[1536:3584, :].rearrange("(k p) b -> p k b", p=128),
                in_=accs[2][:],
            )

    _split_multiwaits(nc)
    return nc


# ---------------- host side ----------------

def _prep_core_inputs(c, x, mask, params, T):
    import ml_dtypes

    bf = ml_dtypes.bfloat16
    b0 = c * BC
    m = {}
    xt = x[b0 : b0 + BC, :T, :].transpose(2, 1, 0).reshape(INS[0], T * BC)
    m["xT"] = np.ascontiguousarray(xt.reshape(KIN[0], 128, T * BC)).astype(bf)
    m["mrep"] = np.ascontiguousarray(
        np.broadcast_to(mask[b0 : b0 + BC, :T].T[:, None, :], (T, 128, BC))
    ).astype(np.float32)
    for l, (Wih, Whh, bias) in enumerate(params):
        H = HS[l]
        m[f"wih{l}"] = np.ascontiguousarray(
            Wih.T.reshape(KIN[l], 128, 3 * H)
        ).astype(bf)
        whhT = Whh.T.reshape(KH[l], 128, H3C[l], 128)
        if l == 0:
            m["whh0"] = np.ascontiguousarray(
                Whh.T.reshape(KH[0], 128, 3 * H)
            ).astype(bf)
            m["b0col"] = np.ascontiguousarray(
                bias.reshape(H3C[0], 128).T
            ).astype(np.float32)
        else:
            m[f"whhblk{l}"] = np.ascontiguousarray(
                whhT.transpose(2, 1, 0, 3)
            ).astype(bf)
            m[f"brep{l}"] = np.ascontiguousarray(
                np.broadcast_to(bias[None, :], (128, 3 * H))
            ).astype(np.float32)
    return m


_cache = {}


def kernel(x, mask, Wih0, Whh0, b0, Wih1, Whh1, b1, Wih2, Whh2, b2):
    from concourse.bass_utils import run_bass_kernel_spmd

    x = np.asarray(x, np.float32)
    mask = np.asarray(mask, np.float32)
    T = x.shape[1]
    if T not in _cache:
        _cache[T] = _build(T)
    nc = _cache[T]
    params = [
        (np.asarray(Wih0, np.float32), np.asarray(Whh0, np.float32),
         np.asarray(b0, np.float32)),
        (np.asarray(Wih1, np.float32), np.asarray(Whh1, np.float32),
         np.asarray(b1, np.float32)),
        (np.asarray(Wih2, np.float32), np.asarray(Whh2, np.float32),
         np.asarray(b2, np.float32)),
    ]
    in_maps = [_prep_core_inputs(c, x, mask, params, T) for c in range(NCORES)]
    res = run_bass_kernel_spmd(nc, in_maps, core_ids=list(range(NCORES)))
    out = np.zeros((B, 3584), np.float32)
    for c in range(NCORES):
        out[c * BC : (c + 1) * BC, :] = res.results[c]["out"].T
    return out
